# revision 9
# baseline (speedup 1.0000x reference)
"""Local softmax attention (GNN message passing) on 8 Trainium2 NeuronCores.

Math (per batch b, node n):
  q/k/v = x @ W{q,k,v}.T + b{q,k,v}              [N, 128], 8 heads x d=16
  scores[n,k,h] = sum_d q[n,h,d] * k[nbr(n,k),h,d] / sqrt(d)
  attn = softmax over k (32 neighbors)
  out[n,h,d] = sum_k attn[n,k,h] * v[nbr(n,k),h,d]

Sharding: 8 cores = (batch b in {0,1}) x (quarter of the node dim).
Each core:
  phase 1: computes k|v (bf16, packed per node row: 512B) for its WHOLE
           batch (redundantly across the 4 cores of that batch) and writes
           them to a private DRAM scratch; computes q (bf16) for its own
           4096 nodes into SBUF.  Projections run on the TensorEngine
           (x^T tiles as stationary operand); bias is added with a rank-1
           matmul (e0 row trick); PSUM->SBUF casts run on the Scalar
           engine (ACT copy).
  phase 2: per 128-node tile: dma_gather (SWDGE) fetches the 32 packed
           k|v neighbor rows per node (4096 x 512B descriptors) into a
           node-major SBUF tile; VectorE computes q*kg, the segmented
           d-reduduction (scores), exp (ScalarE), softmax denominator,
           weighted v aggregation and the 1/Z scaling; result is DMA'd
           out.

SPMD: all 8 cores run the identical program; all per-core variation is
carried by the input data.  Each core's x^T is permuted so its own 4096
nodes come first (tiles 0..31), and the (host-prepared, int16, wrapped)
gather indices are remapped into that permuted row space.
"""

import os
import sys

sys.path.insert(0, "/opt/trn_rl_repo")

from contextlib import ExitStack

import numpy as np

import concourse.bacc as bacc
import concourse.bass as bass
import concourse.tile as tile
from concourse import mybir

HEADS = 8
P = 128


class Cfg:
    def __init__(self, N=16384, K=32, C=128, n_cores=8, B=2):
        self.N, self.K, self.C, self.n_cores, self.B = N, K, C, n_cores, B
        self.quarters = n_cores // B
        self.N_own = N // self.quarters
        self.n_all_tiles = N // P
        self.n_own_tiles = self.N_own // P
        self.d = C // HEADS


def build_nc(cfg: Cfg):
    """Build the (SPMD-uniform) Bass program."""
    N, K, C = cfg.N, cfg.K, cfg.C
    H3 = 3 * C
    f32, bf16, i16 = mybir.dt.float32, mybir.dt.bfloat16, mybir.dt.int16
    T_all, T_own = cfg.n_all_tiles, cfg.n_own_tiles
    d = cfg.d

    nc = bacc.Bacc("TRN2", target_bir_lowering=False, debug=False)

    xt = nc.dram_tensor("xt", [C, N], f32, kind="ExternalInput")
    wqkv = nc.dram_tensor("wqkv", [C, H3], f32, kind="ExternalInput")
    erow = nc.dram_tensor("erow", [C, P], f32, kind="ExternalInput")
    brow = nc.dram_tensor("brow", [C, H3], f32, kind="ExternalInput")
    idxw = nc.dram_tensor("idxw", [P, T_own * (K * P // 16)], i16, kind="ExternalInput")
    outp = nc.dram_tensor("out", [cfg.N_own, C], f32, kind="ExternalOutput")

    with tile.TileContext(nc) as tc, ExitStack() as ctx:
        const = ctx.enter_context(tc.tile_pool(name="const", bufs=1))
        xload = ctx.enter_context(tc.tile_pool(name="xload", bufs=4))
        psum = ctx.enter_context(tc.tile_pool(name="psum", bufs=4, space="PSUM"))
        kvsb = ctx.enter_context(tc.tile_pool(name="kvsb", bufs=4))
        dram = ctx.enter_context(tc.tile_pool(name="dram", bufs=1, space="DRAM"))
        gath = ctx.enter_context(tc.tile_pool(name="gath", bufs=2))
        work = ctx.enter_context(tc.tile_pool(name="work", bufs=2))
        small = ctx.enter_context(tc.tile_pool(name="small", bufs=3))

        # --- constants ---
        wqkv_sb = const.tile([C, H3], f32)
        nc.sync.dma_start(out=wqkv_sb[:], in_=wqkv[:, :])
        erow_sb = const.tile([C, P], f32)
        nc.sync.dma_start(out=erow_sb[:], in_=erow[:, :])
        brow_sb = const.tile([C, H3], f32)
        nc.sync.dma_start(out=brow_sb[:], in_=brow[:, :])
        idx_sb = const.tile([P, T_own * (K * P // 16)], i16)
        nc.sync.dma_start(out=idx_sb[:], in_=idxw[:, :])
        q_all = const.tile([P, T_own * C], bf16)

        kv_dram = dram.tile([N, 2 * C], bf16)

        # --- phase 1: projections ---
        for t in range(T_all):
            xt_t = xload.tile([P, P], f32)
            nc.sync.dma_start(out=xt_t[:], in_=xt[:, t * P : (t + 1) * P])
            pt = psum.tile([P, H3], f32)
            nc.tensor.matmul(out=pt[:], lhsT=xt_t[:], rhs=wqkv_sb[:],
                             start=True, stop=False)
            nc.tensor.matmul(out=pt[:], lhsT=erow_sb[:], rhs=brow_sb[:],
                             start=False, stop=True)
            kv_t = kvsb.tile([P, 2 * C], bf16)
            nc.scalar.activation(out=kv_t[:], in_=pt[:, C:H3],
                                 func=mybir.ActivationFunctionType.Copy)
            nc.sync.dma_start(out=kv_dram[t * P : (t + 1) * P, :], in_=kv_t[:])
            if t < T_own:
                nc.scalar.activation(out=q_all[:, t * C : (t + 1) * C],
                                     in_=pt[:, 0:C],
                                     func=mybir.ActivationFunctionType.Copy)

        # --- phase 2: gather + attention ---
        DBG = int(os.environ.get("KDBG", "5"))
        # dma_gather hangs/crashes above ~1024 idxs per call (HW-measured:
        # 1024 ok, 2048 hangs, 4096 kills the exec unit) -> split each
        # tile's 4096-row gather into 4 calls of 1024.
        NCALL = 4
        ni_call = K * P // NCALL          # 1024
        k_call = K // NCALL               # 8 neighbor slots per call
        cols_call = ni_call // 16         # 64 idx columns per call
        idx_cols = NCALL * cols_call      # per tile
        for t in range(T_own):
            if DBG < 1:
                outt = small.tile([P, C], f32)
                nc.vector.memset(outt[:], 0.0)
                nc.sync.dma_start(out=outp[t * P : (t + 1) * P, :], in_=outt[:])
                continue
            kvg = gath.tile([P, K, 2 * C], bf16)
            for i in range(NCALL):
                c0 = t * idx_cols + i * cols_call
                nc.gpsimd.dma_gather(
                    out_ap=kvg[:, i * k_call : (i + 1) * k_call, :],
                    in_ap=kv_dram[:],
                    idxs_ap=idx_sb[:, c0 : c0 + cols_call],
                    num_idxs=ni_call,
                    num_idxs_reg=ni_call,
                    elem_size=2 * C,
                )

            kg = kvg[:, :, 0:C]            # [P, (k: step 2C), (hd: step 1)]
            vg = kvg[:, :, C : 2 * C]

            if DBG < 2:
                outt = small.tile([P, C], f32)
                nc.vector.tensor_copy(outt[:], kvg[:, 0, 0:C])
                nc.sync.dma_start(out=outp[t * P : (t + 1) * P, :], in_=outt[:])
                continue

            qt = q_all[:, t * C : (t + 1) * C]
            q_b = bass.AP(tensor=qt.tensor, offset=qt.offset,
                          ap=[qt.ap[0], [0, K], [1, C]])

            prod = work.tile([P, K * C], bf16)   # (k, h, d)
            prod_o = prod[:]
            prod_khd = bass.AP(tensor=prod_o.tensor, offset=prod_o.offset,
                               ap=[prod_o.ap[0], [C, K], [1, C]])
            nc.vector.tensor_tensor(out=prod_khd, in0=kg, in1=q_b,
                                    op=mybir.AluOpType.mult)

            # scores[n, (k,h)] = sum_d prod  (f32)
            scores = small.tile([P, K * HEADS], f32)
            s_o = scores[:]
            in_khd = bass.AP(tensor=prod_o.tensor, offset=prod_o.offset,
                             ap=[prod_o.ap[0], [C, K], [d, HEADS], [1, d]])
            out_kh = bass.AP(tensor=s_o.tensor, offset=s_o.offset,
                             ap=[s_o.ap[0], [HEADS, K], [1, HEADS]])
            nc.vector.tensor_reduce(out=out_kh, in_=in_khd,
                                    axis=mybir.AxisListType.X,
                                    op=mybir.AluOpType.add)

            if DBG < 3:
                outt = small.tile([P, C], f32)
                nc.vector.tensor_copy(outt[:], scores[:, 0:C])
                nc.sync.dma_start(out=outp[t * P : (t + 1) * P, :], in_=outt[:])
                continue

            # exps (bf16) = exp(scores / sqrt(d))
            exps = small.tile([P, K * HEADS], bf16)
            nc.scalar.activation(out=exps[:], in_=scores[:],
                                 func=mybir.ActivationFunctionType.Exp,
                                 scale=1.0 / float(np.sqrt(d)))

            # z[n, h] = sum_k exps
            z = small.tile([P, HEADS], f32)
            e_o = exps[:]
            in_hk = bass.AP(tensor=e_o.tensor, offset=e_o.offset,
                            ap=[e_o.ap[0], [1, HEADS], [HEADS, K]])
            nc.vector.tensor_reduce(out=z[:], in_=in_hk,
                                    axis=mybir.AxisListType.X,
                                    op=mybir.AluOpType.add)
            rz = small.tile([P, HEADS], f32)
            nc.vector.reciprocal(rz[:], z[:])

            if DBG < 4:
                outt = small.tile([P, C], f32)
                rz_bb = bass.AP(tensor=rz[:].tensor, offset=rz[:].offset,
                                ap=[rz[:].ap[0], [1, HEADS], [0, d]])
                zc = small.tile([P, C], f32)
                nc.vector.tensor_copy(zc[:], rz_bb)
                nc.vector.tensor_copy(outt[:], zc[:])
                nc.sync.dma_start(out=outp[t * P : (t + 1) * P, :], in_=outt[:])
                continue

            # prod2 = exps (bcast over d) * vg   (k, h, d) bf16
            prod2 = work.tile([P, K * C], bf16)
            p2_o = prod2[:]
            p2_khd = bass.AP(tensor=p2_o.tensor, offset=p2_o.offset,
                             ap=[p2_o.ap[0], [C, K], [1, C]])
            e_khd = bass.AP(tensor=e_o.tensor, offset=e_o.offset,
                            ap=[e_o.ap[0], [HEADS, K], [1, HEADS], [0, d]])
            nc.vector.tensor_tensor(out=p2_khd, in0=vg, in1=e_khd,
                                    op=mybir.AluOpType.mult)

            # acc[n, (h,d)] = sum_k prod2   (k innermost, strided)
            acc = small.tile([P, C], f32)
            in_hdk = bass.AP(tensor=p2_o.tensor, offset=p2_o.offset,
                             ap=[p2_o.ap[0], [d, HEADS], [1, d], [C, K]])
            nc.vector.tensor_reduce(out=acc[:], in_=in_hdk,
                                    axis=mybir.AxisListType.X,
                                    op=mybir.AluOpType.add)

            # out = acc * (1/z) broadcast over d
            outt = small.tile([P, C], f32)
            rz_o = rz[:]
            rz_b = bass.AP(tensor=rz_o.tensor, offset=rz_o.offset,
                           ap=[rz_o.ap[0], [1, HEADS], [0, d]])
            nc.vector.tensor_tensor(out=outt[:], in0=acc[:], in1=rz_b,
                                    op=mybir.AluOpType.mult)
            nc.sync.dma_start(out=outp[t * P : (t + 1) * P, :], in_=outt[:])

    nc.compile()
    return nc


def make_in_maps(cfg: Cfg, x, Wq, bq, Wk, bk, Wv, bv, neighbor_index):
    """Host-side input marshalling: shard, permute (own nodes first),
    build wrapped int16 gather indices."""
    N, K, C = cfg.N, cfg.K, cfg.C
    T_own, N_own = cfg.n_own_tiles, cfg.N_own

    x = np.asarray(x, np.float32)
    wqkv = np.concatenate(
        [np.asarray(Wq, np.float32).T, np.asarray(Wk, np.float32).T,
         np.asarray(Wv, np.float32).T], axis=1)
    wqkv = np.ascontiguousarray(wqkv)
    erow = np.zeros((C, P), np.float32)
    erow[0, :] = 1.0
    brow = np.zeros((C, 3 * C), np.float32)
    brow[0, :] = np.concatenate(
        [np.asarray(bq, np.float32), np.asarray(bk, np.float32),
         np.asarray(bv, np.float32)])
    nbr = np.asarray(neighbor_index, np.int64)

    in_maps = []
    for c in range(cfg.n_cores):
        b, qt = divmod(c, cfg.quarters)
        own = np.arange(qt * N_own, (qt + 1) * N_own)
        rest = np.concatenate(
            [np.arange(0, qt * N_own), np.arange((qt + 1) * N_own, N)])
        perm = np.concatenate([own, rest])
        inv = np.empty(N, np.int64)
        inv[perm] = np.arange(N)

        xt_c = np.ascontiguousarray(x[b].T[:, perm])

        NCALL = 4
        nb = inv[nbr[own]]                                  # [N_own, K]
        vals = nb.reshape(T_own, P, K).transpose(0, 2, 1)   # [T, k, nl]
        # per call: j' = k_local*128 + nl; wrap j' -> [j'%16, j'//16]
        vals = vals.reshape(T_own, NCALL, (K // NCALL) * P)
        a = vals.reshape(T_own, NCALL, (K // NCALL) * P // 16, 16)
        a = a.transpose(3, 0, 1, 2)                          # [16, T, NCALL, S]
        rep = np.tile(a, (8, 1, 1, 1))                       # [128, T, NCALL, S]
        idxw = np.ascontiguousarray(
            rep.reshape(P, T_own * (K * P // 16)).astype(np.int16))

        in_maps.append({
            "xt": xt_c, "wqkv": wqkv, "erow": erow, "brow": brow,
            "idxw": idxw,
        })
    return in_maps


_CACHE = {}


def _get_nc(cfg: Cfg):
    key = (cfg.N, cfg.K, cfg.C, cfg.n_cores, cfg.B)
    if key not in _CACHE:
        _CACHE[key] = build_nc(cfg)
    return _CACHE[key]


def kernel(x, Wq, bq, Wk, bk, Wv, bv, neighbor_index, _trace=False):
    from concourse.bass_utils import run_bass_kernel_spmd

    x = np.asarray(x)
    B, N, C = x.shape
    K = np.asarray(neighbor_index).shape[1]
    cfg = Cfg(N=N, K=K, C=C, n_cores=8, B=B)
    nc = _get_nc(cfg)
    in_maps = make_in_maps(cfg, x, Wq, bq, Wk, bk, Wv, bv, neighbor_index)
    res = run_bass_kernel_spmd(nc, in_maps, core_ids=list(range(cfg.n_cores)),
                               trace=_trace)
    out = np.empty((B, N, C), np.float32)
    for c in range(cfg.n_cores):
        b, qt = divmod(c, cfg.quarters)
        out[b, qt * cfg.N_own : (qt + 1) * cfg.N_own, :] = res.results[c]["out"]
    if _trace:
        kernel.last_results = res
    return out


# revision 11
# speedup vs baseline: 1.3034x; 1.3034x over previous
"""Local softmax attention (GNN message passing) on 8 Trainium2 NeuronCores.

Math (per batch b, node n):
  q/k/v = x @ W{q,k,v}.T + b{q,k,v}              [N, 128], 8 heads x d=16
  scores[n,k,h] = sum_d q[n,h,d] * k[nbr(n,k),h,d] / sqrt(d)
  attn = softmax over k (32 neighbors)
  out[n,h,d] = sum_k attn[n,k,h] * v[nbr(n,k),h,d]

Sharding: 8 cores, each owning a 2048-node range (both batches).

The dominant cost on TRN2 is the neighbor gather: SWDGE descriptor
generation costs ~8.6 ns per gathered row (Q7 software), so k|v for BOTH
batches are packed into one 1KB DRAM row per node and each gathered row
serves both batch instances -> 65536 rows per core.  dma_gather is
limited to ~1024 indices per call (the 128-deep SWDGE descriptor ring:
2048 idxs hangs the exec unit) so each 128-node tile's 4096-row gather is
split into 4 calls.

Phase 1 (projections): every core redundantly computes k|v (bf16) for
all nodes of both batches on the TensorEngine (x^T tiles stationary,
rank-1 e0-row matmul adds the bias) and writes the packed rows to a
private DRAM scratch; q (bf16) for its own nodes stays in SBUF.

Phase 2 (per 128-node tile, per batch): VectorE does q*kg (bf16 2x),
a 4-level pairwise-add tree over d for the scores (last levels fp32),
softmax denominator, attn*vg (bf16 2x, exp expanded over d by ScalarE so
both operands are dense), a 5-level pairwise tree over k, and the 1/Z
scale.  ScalarE does the exp (reading scores with a step-0 broadcast AP
so the output is already expanded over d).

SPMD: all 8 cores run the identical program; per-core variation is data
only (each core's x^T is permuted so its own 2048 nodes come first, and
gather indices are remapped into that row space).
"""

import os
import sys

sys.path.insert(0, "/opt/trn_rl_repo")

from contextlib import ExitStack

import numpy as np

import concourse.bacc as bacc
import concourse.bass as bass
import concourse.tile as tile
from concourse import mybir

HEADS = 8
P = 128
NCALL = 4          # gather calls per tile (1024 idxs each)


class Cfg:
    def __init__(self, N=16384, K=32, C=128, n_cores=8, B=2):
        self.N, self.K, self.C, self.n_cores, self.B = N, K, C, n_cores, B
        self.N_own = N // n_cores
        self.n_all_tiles = N // P
        self.n_own_tiles = self.N_own // P
        self.d = C // HEADS


def _ap(base, dims):
    return bass.AP(tensor=base.tensor, offset=base.offset,
                   ap=[base.ap[0]] + [list(x) for x in dims])


def _off(base, elems):
    return bass.AP(tensor=base.tensor, offset=base.offset + elems,
                   ap=base.ap)


def build_nc(cfg: Cfg):
    N, K, C, B = cfg.N, cfg.K, cfg.C, cfg.B
    H3 = 3 * C
    R = 2 * B * C              # packed row elems (k|v per batch): 512
    f32, bf16, i16 = mybir.dt.float32, mybir.dt.bfloat16, mybir.dt.int16
    T_all, T_own = cfg.n_all_tiles, cfg.n_own_tiles
    d = cfg.d
    ni_call = K * P // NCALL
    k_call = K // NCALL
    cols_call = ni_call // 16
    idx_cols = NCALL * cols_call

    nc = bacc.Bacc("TRN2", target_bir_lowering=False, debug=False)

    xt = nc.dram_tensor("xt", [C, B * N], f32, kind="ExternalInput")
    wqkv = nc.dram_tensor("wqkv", [C, H3], f32, kind="ExternalInput")
    erow = nc.dram_tensor("erow", [C, P], f32, kind="ExternalInput")
    brow = nc.dram_tensor("brow", [C, H3], f32, kind="ExternalInput")
    idxw = nc.dram_tensor("idxw", [P, T_own * idx_cols], i16,
                          kind="ExternalInput")
    outp = nc.dram_tensor("out", [B * cfg.N_own, C], f32,
                          kind="ExternalOutput")

    with tile.TileContext(nc) as tc, ExitStack() as ctx:
        const = ctx.enter_context(tc.tile_pool(name="const", bufs=1))
        xload = ctx.enter_context(tc.tile_pool(name="xload", bufs=4))
        psum = ctx.enter_context(tc.tile_pool(name="psum", bufs=4, space="PSUM"))
        kvsb = ctx.enter_context(tc.tile_pool(name="kvsb", bufs=4))
        dram = ctx.enter_context(tc.tile_pool(name="dram", bufs=1, space="DRAM"))
        gath = ctx.enter_context(tc.tile_pool(name="gath", bufs=2))
        work = ctx.enter_context(tc.tile_pool(name="work", bufs=2))
        small = ctx.enter_context(tc.tile_pool(name="small", bufs=2))

        # --- constants ---
        wqkv_sb = const.tile([C, H3], f32)
        nc.sync.dma_start(out=wqkv_sb[:], in_=wqkv[:, :])
        erow_sb = const.tile([C, P], f32)
        nc.sync.dma_start(out=erow_sb[:], in_=erow[:, :])
        brow_sb = const.tile([C, H3], f32)
        nc.sync.dma_start(out=brow_sb[:], in_=brow[:, :])
        idx_sb = const.tile([P, T_own * idx_cols], i16)
        nc.sync.dma_start(out=idx_sb[:], in_=idxw[:, :])
        q_all = const.tile([P, T_own * B * C], bf16)   # [t][b][hd]

        kv_dram = dram.tile([N, R], bf16)

        # --- phase 1: projections (both batches, all nodes) ---
        for b in range(B):
            for t in range(T_all):
                xt_t = xload.tile([P, P], f32)
                nc.sync.dma_start(out=xt_t[:],
                                  in_=xt[:, b * N + t * P : b * N + (t + 1) * P])
                pt = psum.tile([P, H3], f32)
                nc.tensor.matmul(out=pt[:], lhsT=xt_t[:], rhs=wqkv_sb[:],
                                 start=True, stop=False)
                nc.tensor.matmul(out=pt[:], lhsT=erow_sb[:], rhs=brow_sb[:],
                                 start=False, stop=True)
                kv_t = kvsb.tile([P, 2 * C], bf16)
                nc.scalar.activation(out=kv_t[:], in_=pt[:, C:H3],
                                     func=mybir.ActivationFunctionType.Copy)
                nc.sync.dma_start(
                    out=kv_dram[t * P : (t + 1) * P, 2 * b * C : 2 * (b + 1) * C],
                    in_=kv_t[:])
                if t < T_own:
                    q_slot = (t * B + b) * C
                    nc.scalar.activation(
                        out=q_all[:, q_slot : q_slot + C], in_=pt[:, 0:C],
                        func=mybir.ActivationFunctionType.Copy)

        # --- phase 2: gather + attention ---
        for t in range(T_own):
            kvg = gath.tile([P, K, R], bf16)
            for i in range(NCALL):
                c0 = t * idx_cols + i * cols_call
                nc.gpsimd.dma_gather(
                    out_ap=kvg[:, i * k_call : (i + 1) * k_call, :],
                    in_ap=kv_dram[:],
                    idxs_ap=idx_sb[:, c0 : c0 + cols_call],
                    num_idxs=ni_call,
                    num_idxs_reg=ni_call,
                    elem_size=R,
                )

            for b in range(B):
                kg = kvg[:, :, 2 * b * C : 2 * b * C + C]        # (k, hd)
                vg = kvg[:, :, 2 * b * C + C : 2 * (b + 1) * C]
                qt = q_all[:, (t * B + b) * C : (t * B + b + 1) * C]

                # prod[(k,h,d)] = kg * q   (bf16 2x)
                prod = work.tile([P, K * C], bf16)
                nc.vector.tensor_tensor(
                    out=_ap(prod[:], [[C, K], [1, C]]),
                    in0=kg, in1=_ap(qt, [[0, K], [1, C]]),
                    op=mybir.AluOpType.mult)

                # scores = sum_d prod : 4-level pairwise tree over d
                # L1: (k,h,8) bf16, L2: (k,h,4) bf16, L3: (k,h,2) f32,
                # L4: (k,h) f32
                st1 = small.tile([P, K * HEADS * 8], bf16)
                nc.vector.tensor_tensor(
                    out=_ap(st1[:], [[8, K * HEADS], [1, 8]]),
                    in0=_ap(prod[:], [[d, K * HEADS], [1, 8]]),
                    in1=_ap(_off(prod[:], 8), [[d, K * HEADS], [1, 8]]),
                    op=mybir.AluOpType.add)
                st2 = small.tile([P, K * HEADS * 4], bf16)
                nc.vector.tensor_tensor(
                    out=_ap(st2[:], [[4, K * HEADS], [1, 4]]),
                    in0=_ap(st1[:], [[8, K * HEADS], [1, 4]]),
                    in1=_ap(_off(st1[:], 4), [[8, K * HEADS], [1, 4]]),
                    op=mybir.AluOpType.add)
                st3 = small.tile([P, K * HEADS * 2], f32)
                nc.vector.tensor_tensor(
                    out=_ap(st3[:], [[2, K * HEADS], [1, 2]]),
                    in0=_ap(st2[:], [[4, K * HEADS], [1, 2]]),
                    in1=_ap(_off(st2[:], 2), [[4, K * HEADS], [1, 2]]),
                    op=mybir.AluOpType.add)
                scores = small.tile([P, K * HEADS], f32)       # (k, h)
                nc.vector.tensor_tensor(
                    out=_ap(scores[:], [[1, K * HEADS]]),
                    in0=_ap(st3[:], [[2, K * HEADS]]),
                    in1=_ap(_off(st3[:], 1), [[2, K * HEADS]]),
                    op=mybir.AluOpType.add)

                # expx[(k,h,d)] = exp(scores/4) expanded over d (ScalarE)
                expx = work.tile([P, K * C], bf16)
                nc.scalar.activation(
                    out=_ap(expx[:], [[C, K], [d, HEADS], [1, d]]),
                    in_=_ap(scores[:], [[HEADS, K], [1, HEADS], [0, d]]),
                    func=mybir.ActivationFunctionType.Exp,
                    scale=1.0 / float(np.sqrt(d)))

                # z[h] = sum_k expx[k,h,0]
                z = small.tile([P, HEADS], f32)
                nc.vector.tensor_reduce(
                    out=z[:],
                    in_=_ap(expx[:], [[d, HEADS], [C, K]]),
                    axis=mybir.AxisListType.X, op=mybir.AluOpType.add)
                rz = small.tile([P, HEADS], f32)
                nc.vector.reciprocal(rz[:], z[:])

                # prod2 = expx * vg  (bf16 2x, both dense)
                prod2 = work.tile([P, K * C], bf16)
                nc.vector.tensor_tensor(
                    out=_ap(prod2[:], [[C, K], [1, C]]),
                    in0=vg, in1=_ap(expx[:], [[C, K], [1, C]]),
                    op=mybir.AluOpType.mult)

                # acc = sum_k prod2 : 5-level pairwise tree over k
                at1 = small.tile([P, K * C // 2], bf16)
                nc.vector.tensor_tensor(
                    out=at1[:], in0=prod2[:, 0 : K * C // 2],
                    in1=prod2[:, K * C // 2 : K * C],
                    op=mybir.AluOpType.add)
                at2 = small.tile([P, K * C // 4], bf16)
                nc.vector.tensor_tensor(
                    out=at2[:], in0=at1[:, 0 : K * C // 4],
                    in1=at1[:, K * C // 4 : K * C // 2],
                    op=mybir.AluOpType.add)
                at3 = small.tile([P, K * C // 8], bf16)
                nc.vector.tensor_tensor(
                    out=at3[:], in0=at2[:, 0 : K * C // 8],
                    in1=at2[:, K * C // 8 : K * C // 4],
                    op=mybir.AluOpType.add)
                at4 = small.tile([P, 2 * C], bf16)
                nc.vector.tensor_tensor(
                    out=at4[:], in0=at3[:, 0 : 2 * C], in1=at3[:, 2 * C : 4 * C],
                    op=mybir.AluOpType.add)
                acc = small.tile([P, C], f32)
                nc.vector.tensor_tensor(
                    out=acc[:], in0=at4[:, 0:C], in1=at4[:, C : 2 * C],
                    op=mybir.AluOpType.add)

                # out = acc * (1/z) broadcast over d
                outt = small.tile([P, C], f32)
                nc.vector.tensor_tensor(
                    out=outt[:], in0=acc[:],
                    in1=_ap(rz[:], [[1, HEADS], [0, d]]),
                    op=mybir.AluOpType.mult)
                nc.sync.dma_start(
                    out=outp[b * cfg.N_own + t * P : b * cfg.N_own + (t + 1) * P, :],
                    in_=outt[:])

    nc.compile()
    return nc


def make_in_maps(cfg: Cfg, x, Wq, bq, Wk, bk, Wv, bv, neighbor_index):
    N, K, C, B = cfg.N, cfg.K, cfg.C, cfg.B
    T_own, N_own = cfg.n_own_tiles, cfg.N_own

    x = np.asarray(x, np.float32)
    wqkv = np.ascontiguousarray(np.concatenate(
        [np.asarray(Wq, np.float32).T, np.asarray(Wk, np.float32).T,
         np.asarray(Wv, np.float32).T], axis=1))
    erow = np.zeros((C, P), np.float32)
    erow[0, :] = 1.0
    brow = np.zeros((C, 3 * C), np.float32)
    brow[0, :] = np.concatenate(
        [np.asarray(bq, np.float32), np.asarray(bk, np.float32),
         np.asarray(bv, np.float32)])
    nbr = np.asarray(neighbor_index, np.int64)
    xtb = np.ascontiguousarray(x.transpose(0, 2, 1))   # [B, C, N]

    in_maps = []
    for c in range(cfg.n_cores):
        own = np.arange(c * N_own, (c + 1) * N_own)
        rest = np.concatenate(
            [np.arange(0, c * N_own), np.arange((c + 1) * N_own, N)])
        perm = np.concatenate([own, rest])
        inv = np.empty(N, np.int64)
        inv[perm] = np.arange(N)

        xt_c = np.ascontiguousarray(
            xtb[:, :, perm].transpose(1, 0, 2).reshape(C, B * N))

        nb = inv[nbr[own]]                                  # [N_own, K]
        vals = nb.reshape(T_own, P, K).transpose(0, 2, 1)   # [T, k, nl]
        vals = vals.reshape(T_own, NCALL, (K // NCALL) * P)
        a = vals.reshape(T_own, NCALL, (K // NCALL) * P // 16, 16)
        a = a.transpose(3, 0, 1, 2)                          # [16, T, NCALL, S]
        rep = np.tile(a, (8, 1, 1, 1))                       # [128, ...]
        idxw = np.ascontiguousarray(
            rep.reshape(P, T_own * (K * P // 16)).astype(np.int16))

        in_maps.append({
            "xt": xt_c, "wqkv": wqkv, "erow": erow, "brow": brow,
            "idxw": idxw,
        })
    return in_maps


_CACHE = {}


def _get_nc(cfg: Cfg):
    key = (cfg.N, cfg.K, cfg.C, cfg.n_cores, cfg.B)
    if key not in _CACHE:
        _CACHE[key] = build_nc(cfg)
    return _CACHE[key]


def kernel(x, Wq, bq, Wk, bk, Wv, bv, neighbor_index, _trace=False):
    from concourse.bass_utils import run_bass_kernel_spmd

    x = np.asarray(x)
    B, N, C = x.shape
    K = np.asarray(neighbor_index).shape[1]
    cfg = Cfg(N=N, K=K, C=C, n_cores=8, B=B)
    nc = _get_nc(cfg)
    in_maps = make_in_maps(cfg, x, Wq, bq, Wk, bk, Wv, bv, neighbor_index)
    res = run_bass_kernel_spmd(nc, in_maps, core_ids=list(range(cfg.n_cores)),
                               trace=_trace)
    out = np.empty((B, N, C), np.float32)
    for c in range(cfg.n_cores):
        o = res.results[c]["out"].reshape(B, cfg.N_own, C)
        out[:, c * cfg.N_own : (c + 1) * cfg.N_own, :] = o
    if _trace:
        kernel.last_results = res
    return out
